# revision 13
# baseline (speedup 1.0000x reference)
"""HOG generator kernel for Trainium2, data-parallel over 8 NeuronCores.

v3 (per-iteration = 2 images x one 112-row half, flat [112, 448] free dim):
  - Full 3x3 Sobel on the PE: 5 PSUM-accumulated matmuls with banded
    113x112 vertical matrices (reflect folded in) against column-shifted
    views of one contiguous [113, 450] X tile (2 images side by side,
    1 pad col each end).  The two image-seam columns of gy get a tiny
    correction matmul; gx's four per-image edge columns are exact zeros
    in the reference and are repaired via memsets on A=gx^2 and w.
  - Orientation binning via the cotangent ratio w = gy * (1/gx)
    (reciprocal_approx_fast, 18 bits is plenty): cumulative masks are
    single-scalar compares fused with the magnitude multiply in one
    scalar_tensor_tensor per bin boundary: (w is_lt cot_k) * magG.
  - 8-col cell pooling: fp16 pairwise adds (8->4->2), final pair summed
    by running the fp16 row-pool matmul twice with PSUM accumulation.
  - Bin histograms = adjacent lane differences (k innermost), device
    L2 normalization; ACT uses only Square/Sqrt/Copy (one act table).
  - Device output (img, 28, 9, 28) fp16; host does the unfold permute.
"""
import math
import sys

import numpy as np

sys.path.insert(0, "/opt/trn_rl_repo")

import concourse.bass as bass
import concourse.bacc as bacc
import concourse.mybir as mybir
from concourse import tile
from concourse.bass_utils import run_bass_kernel_spmd

N_CORES = 8
IMGS_PER_CORE = 16
H = W = 224
NB = 9
F32 = mybir.dt.float32
F16 = mybir.dt.float16
AF = mybir.ActivationFunctionType
OP = mybir.AluOpType
COTS = [1.0 / math.tan(k * math.pi / NB) for k in range(1, NB)]


def _host_constants(weight_x, gaussian_kernel):
    """Derive the device constant tensors from the module inputs."""
    wx = np.asarray(weight_x, np.float32).reshape(3, 3)
    v_s = wx[:, 0].copy()                      # [1,2,1] vertical smooth
    v_d = wx[0, :].copy()                      # [1,0,-1] vertical diff
    g2 = np.asarray(gaussian_kernel, np.float64).reshape(16, 16)
    wt = np.sqrt(np.diag(g2)).astype(np.float32)   # g2[i,j] == wt[i]*wt[j]

    def band(chunk, vec):
        m = np.zeros((113, 112), np.float32)
        for i in range(112):
            for d in range(3):
                if chunk == 0:
                    r = i - 1 + d
                    if r == -1:
                        r = 1
                else:
                    r = i + d
                    if r == 113:
                        r = 111
                m[r, i] += vec[d]
        return m

    blob = np.zeros((113, 899), np.float32)
    for ch in range(2):
        sb = band(ch, v_s)
        db = band(ch, v_d)
        blob[:, ch * 112 + 0:ch * 112 + 112] = sb            # smooth
        blob[:, 224 + ch * 112:224 + ch * 112 + 112] = -sb   # -smooth
        blob[:, 448 + ch * 112:448 + ch * 112 + 112] = db    # diff
        blob[:, 672 + ch * 112:672 + ch * 112 + 112] = 2.0 * db
    blob[0:112, 896] = wt[np.arange(112) % 16] ** 2          # row gauss^2
    blob[:, 897] = 0.0
    blob[:, 898] = 1e-24

    blob2 = np.zeros((112, 462), np.float16)
    blob2[:, 0:448] = np.tile(wt[np.arange(224) % 16], 2)[None, :] \
        .astype(np.float16)
    poolm = np.zeros((112, 14), np.float16)
    for r in range(112):
        poolm[r, r // 8] = 1.0
    blob2[:, 448:462] = poolm
    return {"consts": blob, "consts2": blob2}


def _rep(ap, n, pos=1):
    """Insert a broadcast (step-0) dim of size n into an AP at free pos."""
    l = [list(d) for d in ap.ap]
    l.insert(pos, [0, n])
    return bass.AP(ap.tensor, ap.offset, l)


def _cols(ap, base, stride, n):
    """AP view selecting columns base, base+stride, ... of a 2D tile AP."""
    l = [list(ap.ap[0]), [stride, n]]
    return bass.AP(ap.tensor, ap.offset + base, l)


def build_program(n_img=IMGS_PER_CORE):
    assert n_img % 2 == 0
    nc = bacc.Bacc("TRN2", debug=False)
    x_d = nc.dram_tensor("x", [n_img, 224, 224], F32, kind="ExternalInput").ap()
    c1_d = nc.dram_tensor("consts", [113, 899], F32, kind="ExternalInput").ap()
    c2_d = nc.dram_tensor("consts2", [112, 462], F16, kind="ExternalInput").ap()
    out_d = nc.dram_tensor("out", [n_img, 28, 28, NB], F16,
                           kind="ExternalOutput").ap()
    AX = mybir.AxisListType.X

    with tile.TileContext(nc) as tc:
        with (
            tc.tile_pool(name="const", bufs=1) as cp,
            tc.tile_pool(name="work", bufs=3) as wp,
            tc.tile_pool(name="small", bufs=3) as sp,
            tc.tile_pool(name="psum", bufs=3, space="PSUM") as pp,
            tc.tile_pool(name="psum2", bufs=2, space="PSUM") as pp2,
        ):
            CT = cp.tile([113, 899], F32, tag="CT")
            nc.sync.dma_start(CT[:, :], c1_d)
            CT2 = cp.tile([112, 462], F16, tag="CT2")
            nc.scalar.dma_start(CT2[:, :], c2_d)
            SB = [CT[:, 0:112], CT[:, 112:224]]
            SBn = [CT[:, 224:336], CT[:, 336:448]]
            DB = [CT[:, 448:560], CT[:, 560:672]]
            DB2 = [CT[:, 672:784], CT[:, 784:896]]
            gr2 = CT[0:112, 896:897]
            zb = CT[:, 897:898]
            b24 = CT[:, 898:899]
            gc16 = CT2[:, 0:448]
            poolm16 = CT2[:, 448:462]

            pending = [None]

            def flush_post():
                if pending[0] is None:
                    return
                PS, i0 = pending[0]
                pending[0] = None
                # PS: [14, ch2, img2, 28, NB] fp16, k innermost
                Hh = sp.tile([14, 2, 2, 28, NB], F16, tag="Hh")
                nc.vector.tensor_sub(Hh[:, :, :, :, 0:8], PS[:, :, :, :, 0:8],
                                     PS[:, :, :, :, 1:9])
                nc.vector.tensor_copy(Hh[:, :, :, :, 8], PS[:, :, :, :, 8])
                sqt = sp.tile([14, 2, 2, 28, NB], F16, tag="sqt")
                nc.scalar.activation(sqt[:, :, :, :, :], Hh[:, :, :, :, :],
                                     AF.Square, bias=zb[0:14, 0:1])
                ss = sp.tile([14, 2, 2, 28], F32, tag="ss")
                nc.vector.reduce_sum(ss[:, :, :, :], sqt[:, :, :, :, :],
                                     axis=AX)
                nrm = sp.tile([14, 2, 2, 28], F32, tag="nrm")
                nc.scalar.activation(nrm[:, :, :, :], ss[:, :, :, :], AF.Sqrt,
                                     bias=b24[0:14, 0:1])
                inv = sp.tile([14, 2, 2, 28], F32, tag="inv")
                nc.vector.reciprocal_approx_fast(
                    inv[:, :, :, :].rearrange("p a b c -> p (a b c)"),
                    nrm[:, :, :, :].rearrange("p a b c -> p (a b c)"))
                OUT = sp.tile([14, 2, 2, 28, NB], F16, tag="OUT")
                nc.vector.tensor_mul(OUT[:, :, :, :, :], Hh[:, :, :, :, :],
                                     _rep(inv[:, :, :, :], NB, pos=4))
                for c in range(2):
                    nc.gpsimd.dma_start(
                        out_d[i0:i0 + 2, c * 14:(c + 1) * 14, :, :]
                        .rearrange("i p w k -> p i w k"),
                        OUT[:, c, :, :, :])

            iters = [(i0, ch) for i0 in range(0, n_img, 2)
                     for ch in range(2)]
            prod = {}
            ps_tiles = {}

            def produce(idx):
                i0, ch = iters[idx]
                r0 = 0 if ch == 0 else 111
                X = wp.tile([113, 450], F32, tag="X")
                nc.sync.dma_start(X[:, 1:225], x_d[i0, r0:r0 + 113, :])
                nc.scalar.dma_start(X[:, 225:449],
                                    x_d[i0 + 1, r0:r0 + 113, :])
                # reflect pads: col0 <- col2 (img0), col449 <- col447
                nc.gpsimd.tensor_copy(_cols(X[:, :], 0, 449, 2),
                                      _cols(X[:, :], 2, 445, 2))
                # gy seam fix input: {x223-x225, x226-x224}
                CR = wp.tile([113, 2], F32, tag="CR")
                nc.gpsimd.tensor_sub(CR[:, :], _cols(X[:, :], 223, 3, 2),
                                     _cols(X[:, :], 225, -1, 2))
                # horizontal diff for gx on DVE; its two seam cols are
                # exactly 0 under reflect padding
                D = wp.tile([113, 448], F32, tag="D")
                nc.vector.tensor_sub(D[:, :], X[:, 0:448], X[:, 2:450])
                nc.gpsimd.memset(D[:, 223:225], 0.0)

                gxp = pp.tile([112, 448], F32, tag="gx")
                gyp = pp.tile([112, 448], F32, tag="gy")
                nc.tensor.matmul(gxp[:, :], SB[ch], D[:, :],
                                 start=True, stop=True)
                nc.tensor.matmul(gyp[:, :], DB[ch], X[:, 0:448],
                                 start=True, stop=False)
                nc.tensor.matmul(gyp[:, :], DB[ch], X[:, 2:450],
                                 start=False, stop=False)
                nc.tensor.matmul(gyp[:, 223:225], DB[ch], CR[:, :],
                                 start=False, stop=False,
                                 skip_group_check=True)
                nc.tensor.matmul(gyp[:, :], DB2[ch], X[:, 1:449],
                                 start=False, stop=True,
                                 skip_group_check=True)
                prod[idx] = (gxp, gyp)

            def consume(idx):
                i0, ch = iters[idx]
                gxp, gyp = prod.pop(idx)
                if ch == 0:
                    ps_tiles[i0] = sp.tile([14, 2, 2, 28, NB], F16, name="PS",
                                           tag="PS")
                PS = ps_tiles[i0]

                A = wp.tile([112, 448], F32, tag="A")
                nc.scalar.activation(A[:, :], gxp[:, :], AF.Square,
                                     bias=zb[0:112, 0:1])
                C = wp.tile([112, 448], F32, tag="C")
                nc.scalar.activation(C[:, :], gyp[:, :], AF.Square,
                                     bias=zb[0:112, 0:1])
                S2 = wp.tile([112, 448], F32, tag="S2")
                nc.gpsimd.tensor_add(S2[:, :], A[:, :], C[:, :])
                mg = wp.tile([112, 448], F16, tag="mg")
                nc.scalar.activation(mg[:, :], S2[:, :], AF.Sqrt,
                                     bias=zb[0:112, 0:1], scale=gr2)

                SG = wp.tile([112, NB, 448], F16, tag="SG")
                nc.vector.tensor_mul(SG[:, 0, :], mg[:, :], gc16)

                rgx = wp.tile([112, 448], F32, tag="rgx")
                nc.vector.reciprocal_approx_fast(rgx[:, :], gxp[:, :])
                w = wp.tile([112, 448], F32, tag="w")
                nc.vector.tensor_mul(w[:, :], gyp[:, :], rgx[:, :])
                apw = w[:, :]
                nc.gpsimd.memset(
                    bass.AP(apw.tensor, apw.offset,
                            [list(apw.ap[0]), [224, 2], [223, 2]]),
                    30000.0)

                flush_post()

                for k in range(1, NB):
                    nc.vector.scalar_tensor_tensor(
                        SG[:, k, :], w[:, :], COTS[k - 1],
                        SG[:, 0, :], OP.is_lt, OP.mult)

                sgv = SG[:, :, :].rearrange("p k (c e) -> p k c e", e=8)
                T1 = wp.tile([112, NB, 56, 4], F16, tag="T1")
                nc.vector.tensor_add(T1[:, :, :, :],
                                     sgv[:, :, :, 0:4],
                                     sgv[:, :, :, 4:8])
                T2 = wp.tile([112, NB, 56, 2], F16, tag="T2")
                nc.vector.tensor_add(T2[:, :, :, :],
                                     T1[:, :, :, 0:2],
                                     T1[:, :, :, 2:4])

                Pp = pp2.tile([14, NB, 56], F32, tag="Pp")
                nc.tensor.matmul(Pp[:, :, :], poolm16, T2[:, :, :, 0],
                                 start=True, stop=False)
                nc.tensor.matmul(Pp[:, :, :], poolm16, T2[:, :, :, 1],
                                 start=False, stop=True)
                nc.scalar.activation(
                    PS[:, ch, :, :, :],
                    Pp[:, :, :].rearrange("p k (i c) -> p i c k", i=2),
                    AF.Copy)
                if ch == 1:
                    pending[0] = (PS, i0)
                    del ps_tiles[i0]

            produce(0)
            for idx in range(len(iters)):
                if idx + 1 < len(iters):
                    produce(idx + 1)
                consume(idx)
            flush_post()
    nc.compile()
    return nc


def _install_ntff_shim():
    """Provide antenv.axon_hooks (absent in this image) so trace=True works."""
    import sys as _sys
    if "antenv.axon_hooks" in _sys.modules:
        return
    import contextlib
    import ctypes
    import types

    so_path = "/opt/axon/libaxon_pjrt.so"
    lib = ctypes.CDLL(so_path)
    if not hasattr(lib, "axon_start_nrt_profile"):
        hook = None
    else:
        lib.axon_start_nrt_profile.argtypes = [
            ctypes.POINTER(ctypes.c_int64), ctypes.c_size_t]
        lib.axon_start_nrt_profile.restype = ctypes.c_int64
        lib.axon_stop_nrt_profile.argtypes = [ctypes.c_char_p]
        lib.axon_stop_nrt_profile.restype = ctypes.c_int64

        @contextlib.contextmanager
        def hook(output_dir, device_ids):
            import jax
            jax.devices()
            if device_ids:
                ids = (ctypes.c_int64 * len(device_ids))(*device_ids)
                rc = lib.axon_start_nrt_profile(ids, len(device_ids))
            else:
                rc = lib.axon_start_nrt_profile(None, 0)
            if rc != 0:
                raise RuntimeError(f"axon_start_nrt_profile rc={rc}")
            try:
                yield
            finally:
                n = lib.axon_stop_nrt_profile(str(output_dir).encode())
                print(f"profile: {n} file(s) written to {output_dir}",
                      file=sys.stderr)

    mod = types.ModuleType("antenv.axon_hooks")
    mod._hook = hook
    mod.get_axon_ntff_profile_hook = lambda: mod._hook
    mod.set_axon_ntff_profile_hook = lambda h: setattr(mod, "_hook", h)
    _sys.modules["antenv.axon_hooks"] = mod


_prog_cache = {}


def _get_prog(n_img):
    if n_img not in _prog_cache:
        _prog_cache[n_img] = build_program(n_img)
    return _prog_cache[n_img]


def kernel(x, weight_x, weight_y, gaussian_kernel, _trace=False):
    x = np.ascontiguousarray(np.asarray(x, np.float32).reshape(128, 224, 224))
    consts = _host_constants(weight_x, gaussian_kernel)
    nc = _get_prog(IMGS_PER_CORE)
    in_maps = []
    for c in range(N_CORES):
        m = {"x": x[c * IMGS_PER_CORE:(c + 1) * IMGS_PER_CORE]}
        m.update(consts)
        in_maps.append(m)
    if _trace:
        _install_ntff_shim()
    res = run_bass_kernel_spmd(nc, in_maps, core_ids=list(range(N_CORES)),
                               trace=_trace)
    outs = [r["out"] for r in res.results]            # (16, 28, 28, 9) each
    full = np.concatenate(outs, axis=0).astype(np.float32)
    feat = full.reshape(128, 14, 2, 14, 2, NB)
    feat = feat.transpose(0, 1, 3, 5, 2, 4).reshape(128, 196, NB * 4)
    if _trace:
        return np.ascontiguousarray(feat), res
    return np.ascontiguousarray(feat)


# revision 14
# speedup vs baseline: 1.1493x; 1.1493x over previous
"""HOG generator kernel for Trainium2, data-parallel over 8 NeuronCores.

v3 (per-iteration = 2 images x one 112-row half, flat [112, 448] free dim):
  - Full 3x3 Sobel on the PE: 5 PSUM-accumulated matmuls with banded
    113x112 vertical matrices (reflect folded in) against column-shifted
    views of one contiguous [113, 450] X tile (2 images side by side,
    1 pad col each end).  The two image-seam columns of gy get a tiny
    correction matmul; gx's four per-image edge columns are exact zeros
    in the reference and are repaired via memsets on A=gx^2 and w.
  - Orientation binning via the cotangent ratio w = gy * (1/gx)
    (reciprocal_approx_fast, 18 bits is plenty): cumulative masks are
    single-scalar compares fused with the magnitude multiply in one
    scalar_tensor_tensor per bin boundary: (w is_lt cot_k) * magG.
  - 8-col cell pooling: fp16 pairwise adds (8->4->2), final pair summed
    by running the fp16 row-pool matmul twice with PSUM accumulation.
  - Bin histograms = adjacent lane differences (k innermost), device
    L2 normalization; ACT uses only Square/Sqrt/Copy (one act table).
  - Device output (img, 28, 9, 28) fp16; host does the unfold permute.
"""
import math
import sys

import numpy as np

sys.path.insert(0, "/opt/trn_rl_repo")

import concourse.bass as bass
import concourse.bacc as bacc
import concourse.mybir as mybir
from concourse import tile
from concourse.bass_utils import run_bass_kernel_spmd

N_CORES = 8
IMGS_PER_CORE = 16
H = W = 224
NB = 9
F32 = mybir.dt.float32
F16 = mybir.dt.float16
AF = mybir.ActivationFunctionType
OP = mybir.AluOpType
COTS = [1.0 / math.tan(k * math.pi / NB) for k in range(1, NB)]


def _host_constants(weight_x, gaussian_kernel):
    """Derive the device constant tensors from the module inputs."""
    wx = np.asarray(weight_x, np.float32).reshape(3, 3)
    v_s = wx[:, 0].copy()                      # [1,2,1] vertical smooth
    v_d = wx[0, :].copy()                      # [1,0,-1] vertical diff
    g2 = np.asarray(gaussian_kernel, np.float64).reshape(16, 16)
    wt = np.sqrt(np.diag(g2)).astype(np.float32)   # g2[i,j] == wt[i]*wt[j]

    def band(chunk, vec):
        m = np.zeros((113, 112), np.float32)
        for i in range(112):
            for d in range(3):
                if chunk == 0:
                    r = i - 1 + d
                    if r == -1:
                        r = 1
                else:
                    r = i + d
                    if r == 113:
                        r = 111
                m[r, i] += vec[d]
        return m

    blob = np.zeros((113, 899), np.float32)
    for ch in range(2):
        sb = band(ch, v_s)
        db = band(ch, v_d)
        blob[:, ch * 112 + 0:ch * 112 + 112] = sb            # smooth
        blob[:, 224 + ch * 112:224 + ch * 112 + 112] = -sb   # -smooth
        blob[:, 448 + ch * 112:448 + ch * 112 + 112] = db    # diff
        blob[:, 672 + ch * 112:672 + ch * 112 + 112] = 2.0 * db
    blob[0:112, 896] = wt[np.arange(112) % 16] ** 2          # row gauss^2
    blob[:, 897] = 0.0
    blob[:, 898] = 1e-24

    blob2 = np.zeros((112, 462), np.float16)
    blob2[:, 0:448] = np.tile(wt[np.arange(224) % 16], 2)[None, :] \
        .astype(np.float16)
    poolm = np.zeros((112, 14), np.float16)
    for r in range(112):
        poolm[r, r // 8] = 1.0
    blob2[:, 448:462] = poolm
    return {"consts": blob, "consts2": blob2}


def _rep(ap, n, pos=1):
    """Insert a broadcast (step-0) dim of size n into an AP at free pos."""
    l = [list(d) for d in ap.ap]
    l.insert(pos, [0, n])
    return bass.AP(ap.tensor, ap.offset, l)


def _cols(ap, base, stride, n):
    """AP view selecting columns base, base+stride, ... of a 2D tile AP."""
    l = [list(ap.ap[0]), [stride, n]]
    return bass.AP(ap.tensor, ap.offset + base, l)


def build_program(n_img=IMGS_PER_CORE):
    assert n_img % 2 == 0
    nc = bacc.Bacc("TRN2", debug=False)
    x_d = nc.dram_tensor("x", [n_img, 224, 224], F32, kind="ExternalInput").ap()
    c1_d = nc.dram_tensor("consts", [113, 899], F32, kind="ExternalInput").ap()
    c2_d = nc.dram_tensor("consts2", [112, 462], F16, kind="ExternalInput").ap()
    out_d = nc.dram_tensor("out", [n_img, 28, 28, NB], F16,
                           kind="ExternalOutput").ap()
    AX = mybir.AxisListType.X

    with tile.TileContext(nc) as tc:
        with (
            tc.tile_pool(name="const", bufs=1) as cp,
            tc.tile_pool(name="work", bufs=3) as wp,
            tc.tile_pool(name="small", bufs=3) as sp,
            tc.tile_pool(name="psum", bufs=3, space="PSUM") as pp,
            tc.tile_pool(name="psum2", bufs=2, space="PSUM") as pp2,
        ):
            CT = cp.tile([113, 899], F32, tag="CT")
            nc.sync.dma_start(CT[:, :], c1_d)
            CT2 = cp.tile([112, 462], F16, tag="CT2")
            nc.scalar.dma_start(CT2[:, :], c2_d)
            SB = [CT[:, 0:112], CT[:, 112:224]]
            SBn = [CT[:, 224:336], CT[:, 336:448]]
            DB = [CT[:, 448:560], CT[:, 560:672]]
            DB2 = [CT[:, 672:784], CT[:, 784:896]]
            gr2 = CT[0:112, 896:897]
            zb = CT[:, 897:898]
            b24 = CT[:, 898:899]
            gc16 = CT2[:, 0:448]
            poolm16 = CT2[:, 448:462]

            pending = [None]

            def flush_post():
                if pending[0] is None:
                    return
                PS, i0 = pending[0]
                pending[0] = None
                # PS: [14, ch2, img2, 28, NB] fp16, k innermost
                Hh = sp.tile([14, 2, 2, 28, NB], F16, tag="Hh")
                nc.vector.tensor_sub(Hh[:, :, :, :, 0:8], PS[:, :, :, :, 0:8],
                                     PS[:, :, :, :, 1:9])
                nc.vector.tensor_copy(Hh[:, :, :, :, 8], PS[:, :, :, :, 8])
                sqt = sp.tile([14, 2, 2, 28, NB], F16, tag="sqt")
                nc.scalar.activation(sqt[:, :, :, :, :], Hh[:, :, :, :, :],
                                     AF.Square, bias=zb[0:14, 0:1])
                ss = sp.tile([14, 2, 2, 28], F32, tag="ss")
                nc.vector.reduce_sum(ss[:, :, :, :], sqt[:, :, :, :, :],
                                     axis=AX)
                nrm = sp.tile([14, 2, 2, 28], F32, tag="nrm")
                nc.scalar.activation(nrm[:, :, :, :], ss[:, :, :, :], AF.Sqrt,
                                     bias=b24[0:14, 0:1])
                inv = sp.tile([14, 2, 2, 28], F32, tag="inv")
                nc.vector.reciprocal_approx_fast(
                    inv[:, :, :, :].rearrange("p a b c -> p (a b c)"),
                    nrm[:, :, :, :].rearrange("p a b c -> p (a b c)"))
                OUT = sp.tile([14, 2, 2, 28, NB], F16, tag="OUT")
                nc.vector.tensor_mul(OUT[:, :, :, :, :], Hh[:, :, :, :, :],
                                     _rep(inv[:, :, :, :], NB, pos=4))
                for c in range(2):
                    nc.gpsimd.dma_start(
                        out_d[i0:i0 + 2, c * 14:(c + 1) * 14, :, :]
                        .rearrange("i p w k -> p i w k"),
                        OUT[:, c, :, :, :])

            iters = [(i0, ch) for i0 in range(0, n_img, 2)
                     for ch in range(2)]
            prod = {}
            ps_tiles = {}

            def produce(idx):
                i0, ch = iters[idx]
                r0 = 0 if ch == 0 else 111
                X = wp.tile([113, 450], F32, tag="X")
                nc.sync.dma_start(X[:, 1:225], x_d[i0, r0:r0 + 113, :])
                nc.scalar.dma_start(X[:, 225:449],
                                    x_d[i0 + 1, r0:r0 + 113, :])
                # reflect pads: col0 <- col2 (img0), col449 <- col447
                nc.vector.tensor_copy(_cols(X[:, :], 0, 449, 2),
                                      _cols(X[:, :], 2, 445, 2))
                # gy seam fix input: {x223-x225, x226-x224}
                CR = wp.tile([113, 2], F32, tag="CR")
                nc.vector.tensor_sub(CR[:, :], _cols(X[:, :], 223, 3, 2),
                                     _cols(X[:, :], 225, -1, 2))
                # horizontal diff for gx on DVE; its two seam cols are
                # exactly 0 under reflect padding
                D = wp.tile([113, 448], F32, tag="D")
                nc.vector.tensor_sub(D[:, :], X[:, 0:448], X[:, 2:450])
                nc.scalar.memzero(D[:, 223:225])

                gxp = pp.tile([112, 448], F32, tag="gx")
                gyp = pp.tile([112, 448], F32, tag="gy")
                nc.tensor.matmul(gxp[:, :], SB[ch], D[:, :],
                                 start=True, stop=True)
                nc.tensor.matmul(gyp[:, :], DB[ch], X[:, 0:448],
                                 start=True, stop=False)
                nc.tensor.matmul(gyp[:, :], DB[ch], X[:, 2:450],
                                 start=False, stop=False)
                nc.tensor.matmul(gyp[:, 223:225], DB[ch], CR[:, :],
                                 start=False, stop=False,
                                 skip_group_check=True)
                nc.tensor.matmul(gyp[:, :], DB2[ch], X[:, 1:449],
                                 start=False, stop=True,
                                 skip_group_check=True)
                prod[idx] = (gxp, gyp)

            def consume(idx):
                i0, ch = iters[idx]
                gxp, gyp = prod.pop(idx)
                if ch == 0:
                    ps_tiles[i0] = sp.tile([14, 2, 2, 28, NB], F16, name="PS",
                                           tag="PS")
                PS = ps_tiles[i0]

                A = wp.tile([112, 448], F32, tag="A")
                nc.scalar.activation(A[:, :], gxp[:, :], AF.Square,
                                     bias=zb[0:112, 0:1])
                C = wp.tile([112, 448], F32, tag="C")
                nc.scalar.activation(C[:, :], gyp[:, :], AF.Square,
                                     bias=zb[0:112, 0:1])
                S2 = wp.tile([112, 448], F32, tag="S2")
                nc.gpsimd.tensor_add(S2[:, :], A[:, :], C[:, :])
                mg = wp.tile([112, 448], F16, tag="mg")
                nc.scalar.activation(mg[:, :], S2[:, :], AF.Sqrt,
                                     bias=zb[0:112, 0:1], scale=gr2)

                SG = wp.tile([112, NB, 448], F16, tag="SG")
                nc.vector.tensor_mul(SG[:, 0, :], mg[:, :], gc16)

                rgx = wp.tile([112, 448], F32, tag="rgx")
                nc.vector.reciprocal_approx_fast(rgx[:, :], gxp[:, :])
                w = wp.tile([112, 448], F32, tag="w")
                nc.vector.tensor_mul(w[:, :], gyp[:, :], rgx[:, :])
                apw = w[:, :]
                wpatch = bass.AP(apw.tensor, apw.offset,
                                 [list(apw.ap[0]), [224, 2], [223, 2]])
                nc.scalar.activation(wpatch, wpatch, AF.Copy,
                                     bias=30000.0, scale=0.0)

                flush_post()

                for k in range(1, NB):
                    nc.vector.scalar_tensor_tensor(
                        SG[:, k, :], w[:, :], COTS[k - 1],
                        SG[:, 0, :], OP.is_lt, OP.mult)

                sgv = SG[:, :, :].rearrange("p k (c e) -> p k c e", e=8)
                T1 = wp.tile([112, NB, 56, 4], F16, tag="T1")
                nc.vector.tensor_add(T1[:, :, :, :],
                                     sgv[:, :, :, 0:4],
                                     sgv[:, :, :, 4:8])
                T2 = wp.tile([112, NB, 56, 2], F16, tag="T2")
                nc.vector.tensor_add(T2[:, :, :, :],
                                     T1[:, :, :, 0:2],
                                     T1[:, :, :, 2:4])

                Pp = pp2.tile([14, NB, 56], F32, tag="Pp")
                nc.tensor.matmul(Pp[:, :, :], poolm16, T2[:, :, :, 0],
                                 start=True, stop=False)
                nc.tensor.matmul(Pp[:, :, :], poolm16, T2[:, :, :, 1],
                                 start=False, stop=True)
                nc.scalar.activation(
                    PS[:, ch, :, :, :],
                    Pp[:, :, :].rearrange("p k (i c) -> p i c k", i=2),
                    AF.Copy)
                if ch == 1:
                    pending[0] = (PS, i0)
                    del ps_tiles[i0]

            produce(0)
            produce(1)
            for idx in range(len(iters)):
                if idx + 2 < len(iters):
                    produce(idx + 2)
                consume(idx)
            flush_post()
    nc.compile()
    return nc


def _install_ntff_shim():
    """Provide antenv.axon_hooks (absent in this image) so trace=True works."""
    import sys as _sys
    if "antenv.axon_hooks" in _sys.modules:
        return
    import contextlib
    import ctypes
    import types

    so_path = "/opt/axon/libaxon_pjrt.so"
    lib = ctypes.CDLL(so_path)
    if not hasattr(lib, "axon_start_nrt_profile"):
        hook = None
    else:
        lib.axon_start_nrt_profile.argtypes = [
            ctypes.POINTER(ctypes.c_int64), ctypes.c_size_t]
        lib.axon_start_nrt_profile.restype = ctypes.c_int64
        lib.axon_stop_nrt_profile.argtypes = [ctypes.c_char_p]
        lib.axon_stop_nrt_profile.restype = ctypes.c_int64

        @contextlib.contextmanager
        def hook(output_dir, device_ids):
            import jax
            jax.devices()
            if device_ids:
                ids = (ctypes.c_int64 * len(device_ids))(*device_ids)
                rc = lib.axon_start_nrt_profile(ids, len(device_ids))
            else:
                rc = lib.axon_start_nrt_profile(None, 0)
            if rc != 0:
                raise RuntimeError(f"axon_start_nrt_profile rc={rc}")
            try:
                yield
            finally:
                n = lib.axon_stop_nrt_profile(str(output_dir).encode())
                print(f"profile: {n} file(s) written to {output_dir}",
                      file=sys.stderr)

    mod = types.ModuleType("antenv.axon_hooks")
    mod._hook = hook
    mod.get_axon_ntff_profile_hook = lambda: mod._hook
    mod.set_axon_ntff_profile_hook = lambda h: setattr(mod, "_hook", h)
    _sys.modules["antenv.axon_hooks"] = mod


_prog_cache = {}


def _get_prog(n_img):
    if n_img not in _prog_cache:
        _prog_cache[n_img] = build_program(n_img)
    return _prog_cache[n_img]


def kernel(x, weight_x, weight_y, gaussian_kernel, _trace=False):
    x = np.ascontiguousarray(np.asarray(x, np.float32).reshape(128, 224, 224))
    consts = _host_constants(weight_x, gaussian_kernel)
    nc = _get_prog(IMGS_PER_CORE)
    in_maps = []
    for c in range(N_CORES):
        m = {"x": x[c * IMGS_PER_CORE:(c + 1) * IMGS_PER_CORE]}
        m.update(consts)
        in_maps.append(m)
    if _trace:
        _install_ntff_shim()
    res = run_bass_kernel_spmd(nc, in_maps, core_ids=list(range(N_CORES)),
                               trace=_trace)
    outs = [r["out"] for r in res.results]            # (16, 28, 28, 9) each
    full = np.concatenate(outs, axis=0).astype(np.float32)
    feat = full.reshape(128, 14, 2, 14, 2, NB)
    feat = feat.transpose(0, 1, 3, 5, 2, 4).reshape(128, 196, NB * 4)
    if _trace:
        return np.ascontiguousarray(feat), res
    return np.ascontiguousarray(feat)
